# revision 1
# baseline (speedup 1.0000x reference)
"""Absorbed-MLA Bass kernel for 8 trn2 cores, token-sharded down-proj.

Sharding: DP=2 over batch x TP=4 over heads (2 heads/core).
core c -> batch b=c//4, head-group g=c%4 (global heads 2g, 2g+1).

MLA absorption: scores_h = (c_q @ M_h + m0_h)·c_kv + q_rot·k_rot with
M_h = W_uq_h @ W_uk_h^T (b_uk drops: constant-over-keys shifts cancel in
softmax).  out = sum_h softmax_h @ c_kv @ U_h + b_o' with U_h = W_uv_h @
W_o_h and b_o' = b_o + sum_h b_uv_h @ W_o_h.

Per-core device pipeline:
  A: down-proj for THIS RANK'S 512-token slice only, token-major:
     a_tm[tok,288] = h_slice @ [W_dkv|W_dq|W_kr_local]  (+bias via DVE)
  AG0: AllGather a_tm over the 4 TP ranks -> [2048, 288]
  T: transposes give c_kvT/c_qT feat-major + krotT, ckv_tm stays tok-major
  B: q_effT[128,2048] = M_h^T @ c_qT (+m0*scale);  q_rotT[16,2048]+RoPE
  C: S^T[k,q] = c_kvT(k)ᵀ q_effT + k_rotᵀ q_rotT -> exp ->
     latT += ckv_tm(k)ᵀ es;  Z += 1ᵀ es  (no-max softmax)
  AG: AllGather normalized latT [256,2048] bf16 over the 4 TP ranks
  E: out[tok, e_slice] = lat_allᵀᵀ @ U[:, e_slice] + b_o'   (bf16)
Host gathers the 8 [2048,1280] slices into [2,2048,5120].
"""
import sys
if '/opt/trn_rl_repo' not in sys.path:
    sys.path.insert(0, '/opt/trn_rl_repo')
import numpy as np
import ml_dtypes

D_MODEL = 5120
N_HEADS = 8
D_HEAD = 640
D_ROPE = 16
SPLIT = 624
D_KV = 128
BATCH, SEQ = 2, 2048
ROPE_SCALE = 40.0
SCALE = 1.0 / np.sqrt(np.float32(D_HEAD))

N_CORES = 8
TP = 4           # head-parallel ranks per batch group
HL = 2           # local heads per core
ESL = D_MODEL // TP          # 1280 output cols per core
DM_CH = D_MODEL // 128       # 40
NROT = N_HEADS * D_ROPE      # 128 rot feats, ALL heads (rank-uniform)
NKRS = 64                    # swapped+sign-baked kr active dims
NDOWN = 2 * D_KV + NROT + NKRS   # 448 down-proj cols (ckv,cq,kr,kr_sw)
TOKT = SEQ // 128            # 16 token tiles
TOKC = SEQ // 512            # 4 token chunks
LATF = N_HEADS * D_KV        # 1024 latent feats after AllGather

_CACHE = {}
LAST_RESULTS = None
DEBUG_DUMP = False


def _build_nc(with_ag=True):
    import concourse.bacc as bacc
    import concourse.mybir as mybir
    import concourse.tile as tile

    f32 = mybir.dt.float32
    f32r = mybir.dt.float32r
    bf16 = mybir.dt.bfloat16
    Exp = mybir.ActivationFunctionType.Exp
    Ident = mybir.ActivationFunctionType.Identity

    nc = bacc.Bacc("TRN2", target_bir_lowering=False, debug=False,
                   num_devices=N_CORES)

    # pre-tiled on host: col block d holds DRAM rows d*128..d*128+128
    hT = nc.dram_tensor("hT", [128, DM_CH * (SEQ // TP)], bf16,
                        kind="ExternalInput").ap()
    Wdown = nc.dram_tensor("Wdown", [128, DM_CH * NDOWN], bf16,
                           kind="ExternalInput").ap()
    down_bias = nc.dram_tensor("down_bias", [128, NDOWN], f32,
                               kind="ExternalInput").ap()
    Wm = nc.dram_tensor("Wm", [D_KV, HL * 128], f32r, kind="ExternalInput").ap()
    mbias = nc.dram_tensor("mbias", [128, HL], f32, kind="ExternalInput").ap()
    Wqr = nc.dram_tensor("Wqr", [D_KV, HL * NROT], f32r,
                         kind="ExternalInput").ap()
    # swapped+sign-baked q_rot weights: tmp_mm[p] = sign_p*q_rot[(p+4)%8]
    Wqrs = nc.dram_tensor("Wqrs", [D_KV, HL * 64], f32r,
                          kind="ExternalInput").ap()
    qrbias = nc.dram_tensor("qrbias", [128, HL], f32, kind="ExternalInput").ap()
    qrbias_sw = nc.dram_tensor("qrbias_sw", [64, HL], f32,
                               kind="ExternalInput").ap()
    costab = nc.dram_tensor("costab", [64, SEQ], f32r, kind="ExternalInput").ap()
    sintab = nc.dram_tensor("sintab", [64, SEQ], f32r, kind="ExternalInput").ap()
    sinabs = nc.dram_tensor("sinabs", [64, SEQ], f32r, kind="ExternalInput").ap()
    ones_in = nc.dram_tensor("ones", [128, 1], f32r, kind="ExternalInput").ap()
    ident_in = nc.dram_tensor("ident", [128, 128], f32r,
                              kind="ExternalInput").ap()
    U_in = nc.dram_tensor("U", [128, (LATF // 128) * ESL], bf16,
                          kind="ExternalInput").ap()
    bo = nc.dram_tensor("bo", [128, ESL], f32, kind="ExternalInput").ap()
    out = nc.dram_tensor("out", [SEQ, ESL], f32, kind="ExternalOutput").ap()

    from contextlib import ExitStack
    with tile.TileContext(nc) as tc:
        with ExitStack() as _stk:
            dram = _stk.enter_context(
                tc.tile_pool(name="dram", bufs=1, space="DRAM"))
            _inner = ExitStack()
            cst = _inner.enter_context(tc.tile_pool(name="const", bufs=1))
            cTp = _inner.enter_context(tc.tile_pool(name="cT", bufs=1))
            # consts go on the vector/gpsimd queues so they don't delay the
            # phase-A input stream on the sync/scalar queues
            dbias_t = cst.tile([128, NDOWN], f32, name="dbias_t")
            nc.gpsimd.dma_start(dbias_t[:], down_bias)
            mb_t = cst.tile([128, HL], f32, name="mb_t")
            nc.gpsimd.dma_start(mb_t[:], mbias)
            qrb_t = cst.tile([128, HL], f32, name="qrb_t")
            nc.gpsimd.dma_start(qrb_t[:], qrbias)
            ones_t = cst.tile([128, 1], f32r, name="ones_t")
            nc.gpsimd.dma_start(ones_t[:], ones_in)
            ident_t = cst.tile([128, 128], f32r, name="ident_t")
            nc.gpsimd.dma_start(ident_t[:], ident_in)
            wm_t = cst.tile([128, HL * 128], f32r, name="wm_t")
            nc.gpsimd.dma_start(wm_t[:], Wm)
            wqr_t = cst.tile([128, HL * NROT], f32r, name="wqr_t")
            nc.gpsimd.dma_start(wqr_t[:], Wqr)
            wqrs_t = cst.tile([128, HL * 64], f32r, name="wqrs_t")
            nc.gpsimd.dma_start(wqrs_t[:], Wqrs)
            qrbs_t = cst.tile([64, HL], f32, name="qrbs_t")
            nc.gpsimd.dma_start(qrbs_t[:], qrbias_sw)
            cs_t = cst.tile([64, SEQ], f32r, name="cs_t")
            sn_t = cst.tile([64, SEQ], f32r, name="sn_t")
            sq_t = cst.tile([64, SEQ], f32r, name="sq_t")
            nc.gpsimd.dma_start(cs_t[:], costab)
            nc.gpsimd.dma_start(sn_t[:], sintab)
            nc.gpsimd.dma_start(sq_t[:], sinabs)

            c_kvT = cTp.tile([128, SEQ], f32r, name="c_kvT")
            c_qT_c = [cTp.tile([128, 512], f32r, name=f"c_qT{i}")
                      for i in range(TOKC)]
            # token-major [tok, 384] tiles; col slices feed transposes and
            # the PV contraction (cols 0:128 = c_kv) directly
            a_full = [cTp.tile([128, NDOWN], f32r, name=f"afull{t}")
                      for t in range(TOKT)]
            # q_eff per head; rot tensors hold ALL heads' rot feats with the
            # active (rope-rotated) dims packed in rows 0:64, passive 64:128
            qeffT = [cTp.tile([128, SEQ], f32r, name=f"qeffT{l}")
                     for l in range(HL)]
            qrotT = [cTp.tile([128, SEQ], f32r, name=f"qrotT{l}")
                     for l in range(HL)]
            krotC = cTp.tile([128, SEQ], f32r, name="krotC")
            krS = cTp.tile([64, SEQ], f32r, name="krS")

            ag_in = [dram.tile([HL * 128, SEQ // 2], bf16, name=f"ag_in{i}")
                     for i in range(2)]
            ag_out = [dram.tile([LATF, SEQ // 2], bf16, name=f"ag_out{i}")
                      for i in range(2)]
            # AG0 in two token-halves (rank's tok tiles 0,1 / 2,3) so the
            # transpose pipeline starts while the second half gathers
            ag_in0 = [dram.tile([SEQ // TP // 2, NDOWN], f32r,
                                name=f"ag_in0{i}") for i in range(2)]
            ag_out0 = [dram.tile([SEQ // 2, NDOWN], f32r,
                                 name=f"ag_out0{i}") for i in range(2)]

            # ---------- phase A: down-proj, this rank's 512 tokens ----------
            # host pre-tiled hT/Wdown: one big DMA each (128 descriptors)
            with tc.tile_pool(name="wdown", bufs=1) as wdp, \
                 tc.tile_pool(name="atm", bufs=4) as atp, \
                 tc.tile_pool(name="psA", bufs=4, space="PSUM") as psA:
                # separate chunk tiles so dep tracking lets the dm-loop
                # matmuls start as soon as chunk 0 lands
                wd_c, h_c = [], []
                for ch in range(4):
                    w = wdp.tile([128, 10 * NDOWN], bf16, name=f"wd_c{ch}")
                    nc.sync.dma_start(
                        w[:], Wdown[:, ch * 10 * NDOWN:(ch + 1) * 10 * NDOWN])
                    wd_c.append(w)
                    h = wdp.tile([128, 10 * 512], bf16, name=f"h_c{ch}")
                    nc.scalar.dma_start(
                        h[:], hT[:, ch * 10 * 512:(ch + 1) * 10 * 512])
                    h_c.append(h)
                # tt-outer so each token tile finishes early and its AG0
                # half can fire before the rest of phase A completes
                for tt in range(4):
                    pst = psA.tile([128, NDOWN], f32, tag="psA",
                                   name=f"psA_{tt}")
                    for dm in range(DM_CH):
                        ch, dd = divmod(dm, 10)
                        nc.tensor.matmul(
                            pst[:],
                            h_c[ch][:, dd * 512 + tt * 128:dd * 512 + (tt + 1) * 128],
                            wd_c[ch][:, dd * NDOWN:(dd + 1) * NDOWN],
                            start=(dm == 0), stop=(dm == DM_CH - 1),
                            skip_group_check=True)
                    a_tm = atp.tile([128, NDOWN], f32r, tag="atm",
                                    name=f"atm{tt}")
                    nc.vector.tensor_add(a_tm[:], pst[:], dbias_t[:])
                    half, sl2 = divmod(tt, 2)
                    nc.sync.dma_start(
                        ag_in0[half][sl2 * 128:(sl2 + 1) * 128, :], a_tm[:])
                    if tt % 2 == 1:
                        if with_ag:
                            nc.gpsimd.collective_compute(
                                "AllGather", mybir.AluOpType.bypass,
                                replica_groups=[[0, 1, 2, 3], [4, 5, 6, 7]],
                                ins=[ag_in0[half].opt()],
                                outs=[ag_out0[half].opt()])
                        else:  # TimelineSim variant (no collectives)
                            nc.sync.dma_start(
                                ag_out0[half][0:SEQ // TP // 2, :],
                                ag_in0[half][:])

            with tc.tile_pool(name="expp", bufs=9) as expp, \
                 tc.tile_pool(name="ev", bufs=6) as evp, \
                 tc.tile_pool(name="ps_s", bufs=5, space="PSUM") as ps_s, \
                 tc.tile_pool(name="ps_pv", bufs=2, space="PSUM") as ps_pv, \
                 tc.tile_pool(name="ps_z", bufs=1, space="PSUM") as ps_z:
                # ---------- load tok-major tiles; transpose to feat-major ----
                # global token block t: rank r=t//4, tile tt=t%4 lives in
                # ag_out0[tt//2] at row r*256 + (tt%2)*128
                for half in range(2):
                    for r in range(TP):
                        for s2 in range(2):
                            t = 4 * r + half * 2 + s2
                            row = r * 256 + s2 * 128
                            q = nc.sync if t % 2 == 0 else nc.scalar
                            q.dma_start(a_full[t][:],
                                        ag_out0[half][row:row + 128, :])
                # 4 transposes share one PSUM bank -> single batched evac;
                # ckv/kr first so krot's rope overlaps the c_q transposes
                def rope(x, xi, sl=slice(0, SEQ)):
                    # active dims rows 0:64; row h*8+p pairs with h*8+(p+4)%8
                    w = sl.stop - sl.start
                    tmp = expp.tile([64, w], f32r, tag=f"rope{w}",
                                    name=f"rope_{xi}_{sl.start}")
                    for hh in range(N_HEADS):
                        q = nc.sync if hh % 2 == 0 else nc.scalar
                        q.dma_start(tmp[hh * 8:hh * 8 + 4, :],
                                    x[hh * 8 + 4:hh * 8 + 8, sl])
                        q.dma_start(tmp[hh * 8 + 4:hh * 8 + 8, :],
                                    x[hh * 8:hh * 8 + 4, sl])
                    nc.vector.tensor_mul(tmp[0:64, :], tmp[0:64, :],
                                         sn_t[0:64, sl])
                    nc.vector.tensor_mul(x[0:64, sl], x[0:64, sl],
                                         cs_t[0:64, sl])
                    nc.vector.tensor_add(x[0:64, sl], x[0:64, sl],
                                         tmp[0:64, :])

                # batches grouped by AG0 half so half-a transposes overlap
                # the half-b gather; each batch -> two [P,256] evacs
                def tr_batch(src_lo, dst_fn, pw=128):
                    for half in range(2):
                        for rb in range(2):
                            ps4 = ps_s.tile([pw, 512], f32r, tag="s",
                                            name=f"ps4_{src_lo}_{half}_{rb}")
                            for j in range(4):
                                r = 2 * rb + j // 2
                                t = 4 * r + half * 2 + (j % 2)
                                nc.tensor.transpose(
                                    ps4[:, j * 128:(j + 1) * 128],
                                    a_full[t][:, src_lo:src_lo + pw],
                                    ident_t[:])
                            for pj in range(2):
                                r = 2 * rb + pj
                                gcol = 512 * r + half * 256
                                nc.scalar.activation(
                                    dst_fn(gcol, 256),
                                    ps4[:, pj * 256:(pj + 1) * 256], Ident)

                tr_batch(0, lambda c, w: c_kvT[:, c:c + w])
                tr_batch(256, lambda c, w: krotC[:, c:c + w])
                tr_batch(384, lambda c, w: krS[:, c:c + w], pw=64)
                # krot rope via the matmul-free swapped term (no DMAs)
                nc.vector.tensor_mul(krS[:], krS[:], sq_t[:])
                nc.vector.tensor_mul(krotC[0:64, :], krotC[0:64, :], cs_t[:])
                nc.vector.tensor_add(krotC[0:64, :], krotC[0:64, :], krS[:])
                tr_batch(128, lambda c, w: c_qT_c[c // 512][:, c % 512:c % 512 + w])

                # ---------- phase B: q_eff, q_rot + RoPE ----------
                for hl in range(HL):
                    for tc4 in range(TOKC):
                        sl = slice(tc4 * 512, (tc4 + 1) * 512)
                        ps = ps_s.tile([128, 512], f32, tag="s",
                                       name=f"psqe{hl}_{tc4}")
                        nc.tensor.matmul(
                            ps[:], wm_t[:, hl * 128:(hl + 1) * 128],
                            c_qT_c[tc4][:], start=True, stop=True,
                            skip_group_check=True)
                        nc.scalar.activation(qeffT[hl][:, sl], ps[:], Ident,
                                             bias=mb_t[:, hl:hl + 1],
                                             scale=float(SCALE))
                        psr = ps_z.tile([128, 512], f32, tag="z",
                                        name=f"psqr{hl}_{tc4}")
                        nc.tensor.matmul(
                            psr[:], wqr_t[:, hl * NROT:(hl + 1) * NROT],
                            c_qT_c[tc4][:], start=True, stop=True,
                            skip_group_check=True)
                        nc.scalar.activation(qrotT[hl][:, sl], psr[:], Ident,
                                             bias=qrb_t[:, hl:hl + 1],
                                             scale=float(SCALE))
                        # rope via matmul-generated swapped term (no DMAs)
                        psw = ps_z.tile([64, 512], f32, tag="z",
                                        name=f"psw{hl}_{tc4}")
                        nc.tensor.matmul(
                            psw[:], wqrs_t[:, hl * 64:(hl + 1) * 64],
                            c_qT_c[tc4][:], start=True, stop=True,
                            skip_group_check=True)
                        tmq = expp.tile([64, 512], f32r, tag="rope512",
                                        name=f"tmq{hl}_{tc4}")
                        nc.scalar.activation(tmq[:], psw[:], Ident,
                                             bias=qrbs_t[:, hl:hl + 1],
                                             scale=float(SCALE))
                        nc.vector.tensor_mul(tmq[:], tmq[:], sq_t[0:64, sl])
                        nc.vector.tensor_mul(qrotT[hl][0:64, sl],
                                             qrotT[hl][0:64, sl],
                                             cs_t[0:64, sl])
                        nc.vector.tensor_add(qrotT[hl][0:64, sl],
                                             qrotT[hl][0:64, sl], tmq[:])

                if DEBUG_DUMP:
                    dbg_krot = nc.dram_tensor("dbg_krot", [128, SEQ], f32r,
                                              kind="ExternalOutput").ap()
                    dbg_qrot = nc.dram_tensor("dbg_qrot", [128, SEQ], f32r,
                                              kind="ExternalOutput").ap()
                    nc.sync.dma_start(dbg_krot, krotC[:])
                    nc.sync.dma_start(dbg_qrot, qrotT[0][:])

                # ---------- phase C: attention ----------
                # qc outer so each token-half's lat can AllGather while the
                # other half is still computing; Z accumulates on DVE
                for qc in range(TOKC):
                    qsl = slice(qc * 512, (qc + 1) * 512)
                    for hl in range(HL):
                        pv = ps_pv.tile([128, 512], f32, tag="pv",
                                        name=f"pv{hl}_{qc}")
                        z128 = evp.tile([128, 512], f32r, tag="z128",
                                        name=f"z128_{hl}_{qc}")
                        for kt in range(TOKT):
                            ksl = slice(kt * 128, (kt + 1) * 128)
                            psS = ps_s.tile([128, 512], f32, tag="s",
                                            name=f"psS{hl}_{qc}_{kt}")
                            nc.tensor.matmul(
                                psS[:], c_kvT[:, ksl], qeffT[hl][:, qsl],
                                start=True, stop=False, skip_group_check=True)
                            nc.tensor.matmul(
                                psS[:], krotC[:, ksl], qrotT[hl][:, qsl],
                                start=False, stop=True, skip_group_check=True)
                            es = expp.tile([128, 512], f32r, tag="expS",
                                           name=f"es{hl}_{qc}_{kt}")
                            nc.scalar.activation(es[:], psS[:], Exp)
                            nc.tensor.matmul(
                                pv[:], a_full[kt][:, 0:128], es[:],
                                start=(kt == 0), stop=(kt == TOKT - 1),
                                skip_group_check=True)
                            if kt == 0:
                                nc.vector.tensor_copy(z128[:], es[:])
                            else:
                                nc.vector.tensor_add(z128[:], z128[:], es[:])
                        zt = ps_z.tile([1, 512], f32, tag="z",
                                       name=f"z{hl}_{qc}")
                        nc.tensor.matmul(zt[:], ones_t[:], z128[:],
                                         start=True, stop=True,
                                         skip_group_check=True)
                        rz = evp.tile([1, 512], f32, tag="rz",
                                      name=f"rz{hl}_{qc}")
                        nc.vector.reciprocal(rz[:], zt[:])
                        rzb = evp.tile([128, 512], f32, tag="rzb",
                                       name=f"rzb{hl}_{qc}")
                        nc.gpsimd.partition_broadcast(rzb[:], rz[:])
                        ot = evp.tile([128, 512], bf16, tag="ot",
                                      name=f"ot{hl}_{qc}")
                        nc.vector.tensor_mul(ot[:], pv[:], rzb[:])
                        half, qq = divmod(qc, 2)
                        nc.sync.dma_start(
                            ag_in[half][hl * 128:(hl + 1) * 128,
                                        qq * 512:(qq + 1) * 512], ot[:])
                    # fire each half's AllGather as soon as it completes
                    if qc % 2 == 1:
                        half = qc // 2
                        if with_ag:
                            nc.gpsimd.collective_compute(
                                "AllGather", mybir.AluOpType.bypass,
                                replica_groups=[[0, 1, 2, 3], [4, 5, 6, 7]],
                                ins=[ag_in[half].opt()],
                                outs=[ag_out[half].opt()])
                        else:  # TimelineSim variant (no collectives)
                            nc.sync.dma_start(
                                ag_out[half][0:HL * 128, :], ag_in[half][:])

            # ---------- phase E: output projection, per token half ----------
            _inner.close()
            with tc.tile_pool(name="slab", bufs=2) as slp, \
                 tc.tile_pool(name="uw", bufs=1) as uwp, \
                 tc.tile_pool(name="oev", bufs=2) as oev, \
                 tc.tile_pool(name="bo", bufs=1) as bop, \
                 tc.tile_pool(name="psE", bufs=1, space="PSUM") as psE:
                bo_t = bop.tile([128, ESL], f32, name="bo_t")
                nc.sync.dma_start(bo_t[:], bo)
                L_CH = LATF // 128  # 8
                # resident pre-tiled U: col block ki holds U rows ki*128..+128
                u_all = uwp.tile([128, L_CH * ESL], bf16, name="u_all")
                nc.sync.dma_start(u_all[:], U_in)
                # small group last -> short post-matmul store tail
                E_CH = [(0, 512), (512, 512), (1024, 256)]
                oev_bufs = 3  # noqa: F841  (oe ring via tag)
                for th in range(2):
                    slab = []
                    for i in range(L_CH):
                        st = slp.tile([128, SEQ // 2], bf16, tag=f"sl{i}",
                                      name=f"sl{th}_{i}")
                        nc.scalar.dma_start(
                            st[:], ag_out[th][i * 128:(i + 1) * 128, :])
                        slab.append(st)
                    # token-tile pairs: each pair finishes all 3 col groups
                    # then stores its full-width rows -> short final tail
                    for pr in range(4):
                        oe = [oev.tile([128, ESL], f32, tag=f"oe{s}",
                                       name=f"oe_{th}_{pr}_{s}")
                              for s in range(2)]
                        for eoff, ew in E_CH:
                            pse = [psE.tile([128, 512], f32, tag=f"e{s}_{eoff}",
                                            name=f"psE_{th}_{pr}_{eoff}_{s}")
                                   for s in range(2)]
                            for ki in range(L_CH):
                                for s in range(2):
                                    t8 = pr * 2 + s
                                    nc.tensor.matmul(
                                        pse[s][:, :ew],
                                        slab[ki][:, t8 * 128:(t8 + 1) * 128],
                                        u_all[:, ki * ESL + eoff:ki * ESL + eoff + ew],
                                        start=(ki == 0), stop=(ki == L_CH - 1),
                                        skip_group_check=True)
                            for s in range(2):
                                nc.vector.tensor_add(
                                    oe[s][:, eoff:eoff + ew], pse[s][:, :ew],
                                    bo_t[:, eoff:eoff + ew])
                        for s in range(2):
                            tok = th * 8 + pr * 2 + s
                            nc.sync.dma_start(
                                out[tok * 128:(tok + 1) * 128, :], oe[s][:])

    nc.compile()
    return nc


def _rope_tables():
    """[64, SEQ] tables for the packed active-dims layout: row h*8+p holds
    cos/sin for head h's active rope dim p (sin sign-baked per half)."""
    inv_freq = (1.0 / (10000.0 ** (np.arange(0, D_ROPE // 2, 2, dtype=np.float32)
                                   / (D_ROPE // 2)))).astype(np.float32)
    t = np.arange(SEQ, dtype=np.float32) / np.float32(ROPE_SCALE)
    freqs = t[:, None] * inv_freq[None, :]          # [SEQ, 4]
    cos = np.cos(freqs).astype(np.float32).T        # [4, SEQ]
    sin = np.sin(freqs).astype(np.float32).T
    costab = np.empty((64, SEQ), np.float32)
    sintab = np.empty((64, SEQ), np.float32)
    sinabs = np.empty((64, SEQ), np.float32)
    for p in range(8):
        j = p % 4
        costab[np.arange(N_HEADS) * 8 + p] = cos[j]
        sintab[np.arange(N_HEADS) * 8 + p] = -sin[j] if p < 4 else sin[j]
        sinabs[np.arange(N_HEADS) * 8 + p] = sin[j]
    return costab, sintab, sinabs


# rot-feature permutation: row h*8+p <- orig dim h*16+p (active dims of all
# heads packed in rows 0:64), row 64+h*8+p <- orig dim h*16+8+p (passive)
ROT_PERM = np.concatenate(
    [(np.arange(N_HEADS) * D_ROPE)[:, None] + np.arange(8)[None, :],
     (np.arange(N_HEADS) * D_ROPE)[:, None] + 8 + np.arange(8)[None, :]],
    axis=0).reshape(-1)


_FOLD_CACHE = {}


def _fold(W_uq, b_uq, W_uk, W_uv, b_uv, W_o, b_o):
    key = (W_uq.ctypes.data, W_uk.ctypes.data, W_uv.ctypes.data,
           W_o.ctypes.data)
    hit = _FOLD_CACHE.get(key)
    if hit is not None:
        return hit
    M = np.empty((N_HEADS, D_KV, 128), np.float32)
    m0 = np.empty((N_HEADS, 128), np.float32)
    U = np.empty((LATF, D_MODEL), np.float32)
    bo_eff = b_o.astype(np.float32).copy()
    for h in range(N_HEADS):
        Wuq_h = W_uq[:, h * SPLIT:(h + 1) * SPLIT]
        Wuk_h = W_uk[:, h * SPLIT:(h + 1) * SPLIT]
        M[h] = Wuq_h @ Wuk_h.T
        m0[h] = b_uq[h * SPLIT:(h + 1) * SPLIT] @ Wuk_h.T
        Wuv_h = W_uv[:, h * D_HEAD:(h + 1) * D_HEAD]
        Wo_h = W_o[h * D_HEAD:(h + 1) * D_HEAD, :]
        U[h * D_KV:(h + 1) * D_KV] = Wuv_h @ Wo_h
        bo_eff += b_uv[h * D_HEAD:(h + 1) * D_HEAD] @ Wo_h
    U_bf = U.astype(ml_dtypes.bfloat16)
    res = (M, m0, U_bf, bo_eff)
    _FOLD_CACHE[key] = res
    return res


def _shard(inp):
    f32 = np.float32
    h = np.asarray(inp['h'], f32)
    W_dkv = np.asarray(inp['W_dkv'], f32); b_dkv = np.asarray(inp['b_dkv'], f32)
    W_dq = np.asarray(inp['W_dq'], f32); b_dq = np.asarray(inp['b_dq'], f32)
    W_uk = np.asarray(inp['W_uk'], f32); b_uk = np.asarray(inp['b_uk'], f32)
    W_uv = np.asarray(inp['W_uv'], f32); b_uv = np.asarray(inp['b_uv'], f32)
    W_uq = np.asarray(inp['W_uq'], f32); b_uq = np.asarray(inp['b_uq'], f32)
    W_qr = np.asarray(inp['W_qr'], f32); b_qr = np.asarray(inp['b_qr'], f32)
    W_kr = np.asarray(inp['W_kr'], f32); b_kr = np.asarray(inp['b_kr'], f32)
    W_o = np.asarray(inp['W_o'], f32); b_o = np.asarray(inp['b_o'], f32)

    M, m0, U_bf, bo_eff = _fold(W_uq, b_uq, W_uk, W_uv, b_uv, W_o, b_o)
    costab, sintab, sinabs = _rope_tables()
    hTs = [np.ascontiguousarray(h[b].T) for b in range(BATCH)]
    ident = np.eye(128, dtype=f32)
    ones = np.ones((128, 1), f32)

    in_maps = []
    def pretile(a, pr=128):
        """[n*128, w] -> [128, n*w]: col block d holds rows d*128..d*128+128"""
        n = a.shape[0] // pr
        return np.ascontiguousarray(
            a.reshape(n, pr, a.shape[1]).transpose(1, 0, 2).reshape(pr, -1))

    # rank-uniform down-proj weights: kr for ALL heads, rot-permuted,
    # plus the swapped+sign-baked kr active dims for the DMA-free rope
    Wkr_sw = np.zeros((D_MODEL, NKRS), f32)
    bkr_sw = np.zeros(NKRS, f32)
    for hh in range(N_HEADS):
        for p in range(8):
            sgn = -1.0 if p < 4 else 1.0
            src = hh * D_ROPE + (p + 4) % 8
            Wkr_sw[:, hh * 8 + p] = sgn * W_kr[:, src]
            bkr_sw[hh * 8 + p] = sgn * b_kr[src]
    Wdown_u = pretile(np.concatenate(
        [W_dkv, W_dq, W_kr[:, ROT_PERM], Wkr_sw], axis=1).astype(
            ml_dtypes.bfloat16))
    db_row = np.concatenate([b_dkv, b_dq, b_kr[ROT_PERM], bkr_sw])
    db_u = np.ascontiguousarray(np.tile(db_row[None, :], (128, 1)), f32)

    for c in range(N_CORES):
        b, g = divmod(c, TP)
        heads = [2 * g, 2 * g + 1]
        Wm_c = np.concatenate([M[hh] for hh in heads], axis=1)
        mb = np.stack([m0[hh] * SCALE for hh in heads], axis=1)
        # per-head masked q_rot weights in the permuted 128-dim rot space
        Wqr_c = np.zeros((D_KV, HL * NROT), f32)
        qrb = np.zeros((128, HL), f32)
        # swapped+sign-baked variant over the 64 active rows:
        # row h*8+p <- sign_p * orig dim h*16+(p+4)%8
        Wqrs_c = np.zeros((D_KV, HL * 64), f32)
        qrbs = np.zeros((64, HL), f32)
        inv_perm = np.argsort(ROT_PERM)
        for hl, hh in enumerate(heads):
            for j in range(D_ROPE):
                r = inv_perm[hh * D_ROPE + j]
                Wqr_c[:, hl * NROT + r] = W_qr[:, hh * D_ROPE + j]
                qrb[r, hl] = b_qr[hh * D_ROPE + j] * SCALE
            for p in range(8):
                sgn = -1.0 if p < 4 else 1.0
                src = hh * D_ROPE + (p + 4) % 8
                Wqrs_c[:, hl * 64 + hh * 8 + p] = sgn * W_qr[:, src]
                qrbs[hh * 8 + p, hl] = sgn * b_qr[src] * SCALE
        esl = slice(g * ESL, (g + 1) * ESL)
        in_maps.append({
            "hT": pretile(np.ascontiguousarray(
                hTs[b][:, g * (SEQ // TP):(g + 1) * (SEQ // TP)]).astype(
                    ml_dtypes.bfloat16)),
            "Wdown": Wdown_u,
            "down_bias": db_u,
            "Wm": np.ascontiguousarray(Wm_c, f32),
            "mbias": np.ascontiguousarray(mb, f32),
            "Wqr": Wqr_c,
            "Wqrs": Wqrs_c,
            "qrbias": qrb,
            "qrbias_sw": qrbs,
            "costab": costab,
            "sintab": sintab,
            "sinabs": sinabs,
            "ones": ones,
            "ident": ident,
            "U": pretile(np.ascontiguousarray(U_bf[:, esl])),
            "bo": np.ascontiguousarray(
                np.tile(bo_eff[esl][None, :], (128, 1)), f32),
        })
    return in_maps


def kernel(**inputs):
    global LAST_RESULTS
    from concourse import bass_utils
    if 'nc' not in _CACHE:
        _CACHE['nc'] = _build_nc()
    nc = _CACHE['nc']
    in_maps = _shard(inputs)
    res = bass_utils.run_bass_kernel_spmd(nc, in_maps,
                                          core_ids=list(range(N_CORES)))
    LAST_RESULTS = res
    out = np.empty((BATCH, SEQ, D_MODEL), np.float32)
    for c in range(N_CORES):
        b, g = divmod(c, TP)
        out[b, :, g * ESL:(g + 1) * ESL] = res.results[c]["out"]
    return out

